# revision 53
# baseline (speedup 1.0000x reference)
"""MLA prefill kernel for TRN2, 8 NeuronCores.

Sharding: data-parallel over 128-row query blocks. Flattened rows are
[B*S] = 4096 = 2 batches x 16 blocks of 128. Core c (batch b=c//4, j=c%4)
owns blocks {j, 7-j, 8+j, 15-j} of its batch (slots 0..3), so every core
runs an identical program. K^T/V are AllGathered within each batch group
of 4 cores; attention and the output projection run locally.

Attention computes S^T = [key, q] directly (K tile as stationary operand),
so softmax probabilities come out of the exp in the exact layout the AV
matmul consumes - no per-block transposes. The denominator is obtained by
ones-vector matmuls, inverted, and broadcast across partitions with a
rank-1 matmul. Causality: statically skip key blocks above the diagonal
(40 of 64 blocks survive per head), multiplicative {0,1} masks only on the
16 possibly-diagonal (rank, slot) tiles. Heads are software-pipelined so
the tensor engine never waits on the exp.

LayerNorm gamma is folded into the up-projection weights host-side and
beta becomes a per-output-channel bias, applied during PSUM eviction.
Weights stream through SBUF in 256/512-column chunks, one DMA per chunk,
prefetched ahead of the tile-pool release barriers between phases.

All matmuls bf16 (fp32 PSUM); LN statistics fp32; softmax exp f32->bf16.
"""

import math
import os

import numpy as np
import ml_dtypes

import concourse.bass as bass
import concourse.tile as tile
import concourse.mybir as mybir
from concourse import bacc
from concourse.bass_utils import run_bass_kernel_spmd

BF16 = mybir.dt.bfloat16
F32 = mybir.dt.float32
NP_BF16 = ml_dtypes.bfloat16

B, S, D = 2, 2048, 2048
H, DH = 16, 128
P = 128
NCORES = 8
RPC = 512          # rows per core
ROPE_THETA = 10000.0
LN_EPS = 1e-5
V_OFF = H * RPC            # 8192; V region start inside a rank row
KV_COLS = 2 * V_OFF        # 16384

AF = mybir.ActivationFunctionType
ALU = mybir.AluOpType

# (slot, psum/pb column offset, q column offset, width) per key-slot; the
# order packs the four score regions into 3 psum banks without a matmul
# output crossing a bank edge.
SL_LAYOUT = [
    (0, 0, 0, 512),      # sl0 keys: visible to q 0..512
    (1, 512, 128, 384),  # sl1 keys: visible to q 128..512
    (3, 896, 384, 128),  # sl3 keys: visible to q 384..512
    (2, 1024, 256, 256), # sl2 keys: visible to q 256..512
]
PB_W = 1280


def _blocks(c):
    j = c % 4
    return [j, 7 - j, 8 + j, 15 - j]


# ---------------------------------------------------------------- emission


def _emit(nc, tc, t_in, t_out):
    x = t_in["x"].ap()
    wdq = t_in["wdq"].ap()
    wdkv = t_in["wdkv"].ap()
    wuq = t_in["wuq"].ap()
    wk = t_in["wk"].ap()
    wv = t_in["wv"].ap()
    wot = t_in["wot"].ap()
    bias_q = t_in["bias_q"].ap()
    bias_k = t_in["bias_k"].ap()
    biasv = t_in["biasv"].ap()
    gkv = t_in["gkv"].ap()
    bkv = t_in["bkv"].ap()
    cs = t_in["cs"].ap()          # [128, 4, 512] bf16: cosk, sink, cosq, sinq
    masks = t_in["masks"].ap()    # [128, 16, 128] bf16 {0,1}
    out_d = t_out["out"].ap()
    ckv_d = t_out["ckv"].ap()

    with (
        tc.tile_pool(name="consts", bufs=1) as consts,
        tc.tile_pool(name="big", bufs=1) as big,
        tc.tile_pool(name="stat", bufs=24) as stat,
        tc.tile_pool(name="dram", bufs=1, space="DRAM") as dram,
    ):
        ones_col = consts.tile([P, 1], BF16, tag="onec")
        ones_row = consts.tile([1, P], BF16, tag="oner")
        nc.vector.memset(ones_col, 1.0)
        nc.vector.memset(ones_row, 1.0)

        bq_sb = consts.tile([P, H], F32, tag="bq")
        bk_sb = consts.tile([P, H], F32, tag="bk")
        nc.sync.dma_start(bq_sb[:], bias_q[:])
        nc.sync.dma_start(bk_sb[:], bias_k[:])

        qT = big.tile([P, H, RPC], BF16, tag="qT")
        oT = big.tile([P, H, RPC], BF16, tag="oT")
        xT = big.tile([P, 16, RPC], BF16, tag="xT")

        kv_in = dram.tile([P, KV_COLS], BF16)
        no_cc = bool(os.environ.get("BASS_MLA_NO_CC"))
        if no_cc:
            kv_out_r = [
                dram.tile([P, KV_COLS], BF16, name=f"kvo{r}") for r in range(4)
            ]
        else:
            kv_out = dram.tile([4 * P, KV_COLS], BF16)

        # x [512 rows, 2048] -> xT [d_lo=128, d_hi=16, rows=512] via DMA
        # xbar, in four row-chunks so the first down-proj matmuls can start
        # as soon as chunk 0 lands
        for xt in range(4):
            nc.sync.dma_start_transpose(
                xT[:, :, xt * P : (xt + 1) * P],
                x[xt * P : (xt + 1) * P, :].rearrange(
                    "n (po pi) -> n po pi", pi=P
                ),
            )


        def down_ln(w_dram, aT, t_sb, sqd, acc, acc2):
            """t_sb = normalize(x @ W) in place (no gamma/beta), aT = transposed.

            LN staging tiles live in the caller's pool so this function's
            weight pool is the only space reclaimed at exit (its readers
            finish with the matmuls, so the next phase's loads start
            immediately)."""
            with (
                tc.tile_pool(name="wdpool", bufs=3) as wdpool,
                tc.tile_pool(name="dmm", bufs=4, space="PSUM") as dmm,
            ):
                for cc in range(8):
                    w_sb = wdpool.tile([P, 16, 256], BF16, tag="w")
                    nc.sync.dma_start(
                        w_sb[:], w_dram[:, :, cc * 256 : (cc + 1) * 256]
                    )
                    for rt in range(4):
                        ps = dmm.tile([P, 256], F32)
                        for kt in range(16):
                            nc.tensor.matmul(
                                ps,
                                xT[:, kt, rt * P : (rt + 1) * P],
                                w_sb[:, kt, :],
                                start=(kt == 0),
                                stop=(kt == 15),
                            )
                        tslice = t_sb[:, rt, cc * 256 : (cc + 1) * 256]
                        idx = rt * 8 + cc
                        nc.scalar.activation(
                            tslice,
                            ps,
                            AF.Identity,
                            accum_out=acc[:, idx : idx + 1],
                        )
                        nc.vector.tensor_tensor(
                            sqd[:, 0:256], tslice, tslice, ALU.mult
                        )
                        nc.vector.reduce_sum(
                            acc2[:, idx : idx + 1], sqd[:, 0:256],
                            axis=mybir.AxisListType.X,
                        )
                        if cc == 7:
                            # LN for this row-block; overlaps later evictions
                            row = t_sb[:, rt, :]
                            ssum = stat.tile([P, 1], F32, tag="s")
                            nc.vector.reduce_sum(
                                ssum, acc[:, rt * 8 : rt * 8 + 8],
                                axis=mybir.AxisListType.X,
                            )
                            nmu = stat.tile([P, 1], F32, tag="s")
                            nc.vector.tensor_scalar_mul(nmu, ssum, -1.0 / D)
                            ssq = stat.tile([P, 1], F32, tag="s")
                            nc.vector.reduce_sum(
                                ssq, acc2[:, rt * 8 : rt * 8 + 8],
                                axis=mybir.AxisListType.X,
                            )
                            mu2 = stat.tile([P, 1], F32, tag="s")
                            nc.vector.tensor_tensor(mu2, nmu, nmu, ALU.mult)
                            veps = stat.tile([P, 1], F32, tag="s")
                            nc.vector.tensor_scalar(
                                veps, ssq, 1.0 / D, LN_EPS, ALU.mult, ALU.add
                            )
                            nc.vector.tensor_tensor(
                                veps, veps, mu2, ALU.subtract
                            )
                            std = stat.tile([P, 1], F32, tag="s")
                            nc.scalar.activation(std, veps, AF.Sqrt)
                            rstd = stat.tile([P, 1], F32, tag="s")
                            nc.vector.reciprocal(rstd, std)
                            nmurstd = stat.tile([P, 1], F32, tag="s")
                            nc.vector.tensor_tensor(nmurstd, nmu, rstd, ALU.mult)
                            nc.scalar.activation(
                                row, row, AF.Identity,
                                bias=nmurstd, scale=rstd,
                            )
                            nc.scalar.dma_start_transpose(
                                aT[:, :, rt * P : (rt + 1) * P], row
                            )

        def up_rope(w_dram, wpool, aT, bias_t, cos_t, sin_t, dst_fn,
                    done_fn=None, preloaded=None):
            """Per 4-head group g: RoPE((aT^T W_h)^T + bias_h) -> dst_fn(g).

            `preloaded` supplies already-loading chunk tiles for the first
            groups (their DMAs were emitted ahead of the pool-release
            barrier of the preceding phase)."""
            with (
                tc.tile_pool(name="ropest", bufs=2) as ropest,
                tc.tile_pool(name="umm", bufs=2, space="PSUM") as umm,
            ):
                sin_b = sin_t.rearrange("p (o n) -> p o n", o=1).broadcast_to(
                    [P, 4, RPC]
                )
                cos_b = cos_t.rearrange("p (o n) -> p o n", o=1).broadcast_to(
                    [P, 4, RPC]
                )
                for g in range(4):
                    if preloaded is not None and g < len(preloaded):
                        w_sb = preloaded[g]
                    else:
                        w_sb = wpool.tile(
                            [P, 16, 512], BF16, tag="w", name="w_up"
                        )
                        nc.sync.dma_start(
                            w_sb[:], w_dram[:, :, g * 512 : (g + 1) * 512]
                        )
                    raw = ropest.tile([P, 4, RPC], BF16, tag="raw")
                    rot = ropest.tile([P, 4, RPC], BF16, tag="rot")
                    for hh in range(4):
                        h = 4 * g + hh
                        ps = umm.tile([P, RPC], F32)
                        for kt in range(16):
                            nc.tensor.matmul(
                                ps,
                                w_sb[:, kt, hh * P : (hh + 1) * P],
                                aT[:, kt, :],
                                start=(kt == 0),
                                stop=(kt == 15),
                            )
                        nc.scalar.activation(
                            raw[:, hh, :], ps, AF.Identity,
                            bias=bias_t[:, h : h + 1],
                        )
                    # rotate_half: swap dh halves (sign lives in sin_t)
                    nc.scalar.dma_start(rot[0:64, :, :], raw[64:128, :, :])
                    nc.scalar.dma_start(rot[64:128, :, :], raw[0:64, :, :])
                    dst = dst_fn(g)
                    nc.vector.tensor_tensor(rot, rot, sin_b, ALU.mult)
                    nc.vector.tensor_tensor(dst, raw, cos_b, ALU.mult)
                    nc.vector.tensor_tensor(dst, dst, rot, ALU.add)
                    if done_fn is not None:
                        done_fn(g, dst)

        # ================= KV path =================
        # The up-projection weight pool opens before everything else in this
        # phase so its first chunk loads are not gated on the down-proj
        # pool-release barrier (which parks the SP queue until the last
        # down-proj matmul retires).
        with (
            tc.tile_pool(name="upw", bufs=2) as upw,
            tc.tile_pool(name="kvact", bufs=1) as kvact,
        ):
            ckvT = kvact.tile([P, 16, RPC], BF16, tag="aT")
            t_kv = kvact.tile([P, 4, D], BF16, tag="t")
            cs_kv = kvact.tile([P, 2, RPC], BF16, tag="cs")
            nc.sync.dma_start(cs_kv[:], cs[:, 0:2, :])
            sqd_kv = kvact.tile([P, 256], BF16, tag="sqd")
            acc_kv = kvact.tile([P, 32], F32, tag="acc")
            acc2_kv = kvact.tile([P, 32], F32, tag="acc2")
            with tc.tile_pool(name="ckvp", bufs=1) as ckvp:
                # ckv output = t * gamma + beta (bf16); the Pool TTs overlap
                # the K/V up-projections, the store waits until after V.
                g_sb = ckvp.tile([P, D], BF16, tag="g")
                b_sb = ckvp.tile([P, D], BF16, tag="b")
                ckv_sb = ckvp.tile([P, 4, D], BF16, tag="ckv")
                down_ln(wdkv, ckvT, t_kv, sqd_kv, acc_kv, acc2_kv)
                nc.sync.dma_start(g_sb[:], gkv[:])
                nc.sync.dma_start(b_sb[:], bkv[:])
                # prefetch the first K up-proj chunks (fresh space: no wait)
                wk_pre = []
                for g in range(2):
                    w_pre = upw.tile([P, 16, 512], BF16, tag="w", name="w_up")
                    nc.sync.dma_start(
                        w_pre[:], wk[:, :, g * 512 : (g + 1) * 512]
                    )
                    wk_pre.append(w_pre)
                gb = g_sb.rearrange("p (o n) -> p o n", o=1).broadcast_to(
                    [P, 4, D]
                )
                bb = b_sb.rearrange("p (o n) -> p o n", o=1).broadcast_to(
                    [P, 4, D]
                )
                nc.gpsimd.tensor_tensor(ckv_sb, t_kv, gb, ALU.mult)
                nc.gpsimd.tensor_tensor(ckv_sb, ckv_sb, bb, ALU.add)

                # K^T per head + RoPE -> kv_in[:, :8192]
                with (
                    tc.tile_pool(name="vstage", bufs=1) as vstage,
                    tc.tile_pool(name="kout", bufs=1) as kout,
                ):
                    bv_sb = vstage.tile([P, D], BF16, tag="bv")
                    vsb = vstage.tile([P, 4, D], BF16, tag="vsb")
                    ksb = kout.tile([P, H, RPC], BF16, tag="ksb")
                    up_rope(
                        wk, upw, ckvT, bk_sb, cs_kv[:, 0, :], cs_kv[:, 1, :],
                        lambda g: ksb[:, 4 * g : 4 * g + 4, :],
                        preloaded=wk_pre,
                    )
                    # V -> kv_in[:, 8192:]
                    with tc.tile_pool(name="vmm", bufs=2, space="PSUM") as vmm:
                        nc.sync.dma_start(bv_sb[:], biasv[:])
                        for cc in range(4):
                            wv_sb = upw.tile(
                                [P, 16, 512], BF16, tag="w", name="w_up"
                            )
                            nc.sync.dma_start(
                                wv_sb[:], wv[:, :, cc * 512 : (cc + 1) * 512]
                            )
                            if cc == 3:
                                # K store + K-region gather during late V
                                nc.sync.dma_start(kv_in[:, 0:V_OFF], ksb)
                                if no_cc:
                                    for r in range(4):
                                        nc.sync.dma_start(
                                            kv_out_r[r][:, 0:V_OFF],
                                            kv_in[:, 0:V_OFF],
                                        )
                            for slot in range(4):
                                psv = vmm.tile([P, 512], F32)
                                for kt in range(16):
                                    nc.tensor.matmul(
                                        psv,
                                        ckvT[:, kt, slot * P : (slot + 1) * P],
                                        wv_sb[:, kt, :],
                                        start=(kt == 0),
                                        stop=(kt == 15),
                                    )
                                nc.vector.tensor_tensor(
                                    vsb[:, slot, cc * 512 : (cc + 1) * 512],
                                    psv,
                                    bv_sb[:, cc * 512 : (cc + 1) * 512],
                                    ALU.add,
                                )
                    nc.sync.dma_start(kv_in[:, V_OFF:KV_COLS], vsb)



        # rank-0/masks SBUF lives in a pool that reuses KV-phase space, so
        # its load is not blocked behind the Q phase like ranks 1-3
        kv0pool = tc.alloc_tile_pool(name="kv0pool", bufs=1)
        kv_sb0 = kv0pool.tile([P, KV_COLS], BF16, tag="kv0")
        kv_sb1 = kv0pool.tile([P, KV_COLS], BF16, tag="kv1")

        # ================= Q path =================
        with (
            tc.tile_pool(name="wqpool", bufs=2) as wqpool,
            tc.tile_pool(name="qact", bufs=1) as qact,
        ):
            cqT = qact.tile([P, 16, RPC], BF16, tag="aT")
            t_q = qact.tile([P, 4, D], BF16, tag="t")
            cs_q = qact.tile([P, 2, RPC], BF16, tag="cs")
            nc.sync.dma_start(cs_q[:], cs[:, 2:4, :])
            sqd_q = qact.tile([P, 256], BF16, tag="sqd")
            acc_q = qact.tile([P, 32], F32, tag="acc")
            acc2_q = qact.tile([P, 32], F32, tag="acc2")
            down_ln(wdq, cqT, t_q, sqd_q, acc_q, acc2_q)
            wq_pre = []
            for g in range(2):
                w_pre = wqpool.tile([P, 16, 512], BF16, tag="w", name="w_up")
                nc.sync.dma_start(w_pre[:], wuq[:, :, g * 512 : (g + 1) * 512])
                wq_pre.append(w_pre)
            if no_cc:
                for r in range(4):
                    nc.sync.dma_start(
                        kv_out_r[r][:, V_OFF:KV_COLS],
                        kv_in[:, V_OFF:KV_COLS],
                    )

            if not no_cc:
                # AllGather K^T/V within each batch group of 4 cores; fires
                # on the Pool engine as soon as kv_in lands.
                nc.gpsimd.collective_compute(
                    "AllGather",
                    ALU.bypass,
                    replica_groups=[[0, 1, 2, 3], [4, 5, 6, 7]],
                    ins=[kv_in.opt()],
                    outs=[kv_out.opt()],
                )

            up_rope(
                wuq, wqpool, cqT, bq_sb, cs_q[:, 0, :], cs_q[:, 1, :],
                lambda g: qT[:, 4 * g : 4 * g + 4, :],
                preloaded=wq_pre,
            )
            nc.sync.dma_start(
                kv_sb0[:], kv_out_r[0][:] if no_cc else kv_out[0:P, :]
            )
            nc.sync.dma_start(
                kv_sb1[:], kv_out_r[1][:] if no_cc else kv_out[P : 2 * P, :]
            )
            nc.sync.dma_start(
                ckv_d.rearrange("(rt p) n -> p rt n", p=P), ckv_sb
            )

        # ================= attention =================
        with (
            tc.tile_pool(name="kvsb", bufs=1) as kvsb,
            tc.tile_pool(name="pbp", bufs=2) as pbp,
            tc.tile_pool(name="rlp", bufs=1) as rlp,
            tc.tile_pool(name="scp", bufs=2, space="PSUM") as scp,
            tc.tile_pool(name="lrp", bufs=1, space="PSUM") as lrp,
            tc.tile_pool(name="opp", bufs=1, space="PSUM") as opp,
        ):
            mask_sb = kvsb.tile([P, 16, P], BF16, tag="mask")
            nc.sync.dma_start(mask_sb[:], masks[:])
            kv_sb = [kv_sb0, kv_sb1] + [
                kvsb.tile([P, KV_COLS], BF16, tag=f"kv{r}", name=f"kv{r}")
                for r in range(2, 4)
            ]
            for r in range(2, 4):
                src_ap = kv_out_r[r][:] if no_cc else kv_out[r * P : (r + 1) * P, :]
                nc.sync.dma_start(kv_sb[r][:], src_ap)

            def attn_scores(h):
                pb = pbp.tile([P, 4, PB_W], BF16, tag="pb")
                for r in range(4):
                    sc = scp.tile([P, 1536], F32, tag="sc")
                    for sl, pb_off, q_off, n in SL_LAYOUT:
                        nc.tensor.matmul(
                            sc[:, pb_off : pb_off + n],
                            kv_sb[r][:, h * RPC + sl * P : h * RPC + (sl + 1) * P],
                            qT[:, h, q_off:RPC],
                            start=True,
                            stop=True,
                        )
                    nc.scalar.activation(pb[:, r, :], sc[:, 0:PB_W], AF.Exp)
                    for sl, pb_off, q_off, n in SL_LAYOUT:
                        # diagonal (or fully-hidden) tiles: {0,1} mask
                        nc.gpsimd.tensor_tensor(
                            pb[:, r, pb_off : pb_off + P],
                            pb[:, r, pb_off : pb_off + P],
                            mask_sb[:, r * 4 + sl, :],
                            ALU.mult,
                        )
                return pb

            def attn_reduce(h, pb):
                # softmax denominator per q column: ones^T @ P^T
                lr = lrp.tile([P, 512], F32, tag="lr")
                for r in range(4):
                    for i, (sl, pb_off, q_off, n) in enumerate(SL_LAYOUT):
                        nc.tensor.matmul(
                            lr[0:1, q_off:RPC],
                            ones_col,
                            pb[:, r, pb_off : pb_off + n],
                            start=(r == 0 and i == 0),
                            stop=(r == 3 and i == 3),
                        )
                rl = rlp.tile([1, 512], BF16, tag="rl")
                with nc.allow_low_precision(reason="1/l bf16 for bf16 matmul"):
                    nc.vector.reciprocal(rl, lr[0:1, :])
                # O^T = sum V^T P^T
                o_ps = opp.tile([P, 512], F32, tag="o")
                for r in range(4):
                    for i, (sl, pb_off, q_off, n) in enumerate(SL_LAYOUT):
                        nc.tensor.matmul(
                            o_ps[:, q_off:RPC],
                            kv_sb[r][
                                :,
                                V_OFF + sl * 2048 + h * P
                                : V_OFF + sl * 2048 + (h + 1) * P,
                            ],
                            pb[:, r, pb_off : pb_off + n],
                            start=(r == 0 and i == 0),
                            stop=(r == 3 and i == 3),
                        )
                # broadcast 1/l across partitions and normalize; TT may read
                # only one PSUM operand, so stage the broadcast in SBUF
                rlb = lrp.tile([P, 512], F32, tag="lr")
                nc.tensor.matmul(rlb, ones_row, rl, start=True, stop=True)
                rlb_sb = rlp.tile([P, 512], BF16, tag="rlb")
                nc.scalar.activation(rlb_sb, rlb, AF.Identity)
                nc.vector.tensor_tensor(oT[:, h, :], o_ps, rlb_sb, ALU.mult)

            prev = None
            for h in range(H):
                pb = attn_scores(h)
                if prev is not None:
                    attn_reduce(*prev)
                prev = (h, pb)
            attn_reduce(*prev)

        kv0pool.release()

        # ================= output projection =================
        with (
            tc.tile_pool(name="wopool", bufs=2) as wopool,
            tc.tile_pool(name="oev", bufs=3) as oev,
            tc.tile_pool(name="omm", bufs=4, space="PSUM") as omm,
        ):
            for cc in range(4):
                wo_sb = wopool.tile([P, 16, 512], BF16, tag="w")
                nc.sync.dma_start(wo_sb[:], wot[:, :, cc * 512 : (cc + 1) * 512])
                for rt in range(4):
                    ps = omm.tile([P, 512], F32)
                    for h in range(16):
                        nc.tensor.matmul(
                            ps,
                            oT[:, h, rt * P : (rt + 1) * P],
                            wo_sb[:, h, :],
                            start=(h == 0),
                            stop=(h == 15),
                        )
                    o_sb = oev.tile([P, 512], F32, tag="ob")
                    nc.scalar.activation(o_sb, ps, AF.Identity)
                    nc.scalar.dma_start(
                        out_d[rt * P : (rt + 1) * P, cc * 512 : (cc + 1) * 512],
                        o_sb,
                    )


# ---------------------------------------------------------------- build


_CACHE = {}


def _build():
    key = "nocc" if os.environ.get("BASS_MLA_NO_CC") else "cc"
    if key in _CACHE:
        return _CACHE[key]
    nc = bacc.Bacc("TRN2", target_bir_lowering=False, debug=False, num_devices=NCORES)
    t_in = {}

    def inp(name, shape, dt):
        t_in[name] = nc.dram_tensor(name, shape, dt, kind="ExternalInput")

    inp("x", [RPC, D], BF16)
    inp("wdq", [P, 16, D], BF16)
    inp("wdkv", [P, 16, D], BF16)
    inp("wuq", [P, 16, D], BF16)
    inp("wk", [P, 16, D], BF16)
    inp("wv", [P, 16, D], BF16)
    inp("wot", [P, 16, D], BF16)
    inp("bias_q", [P, H], F32)
    inp("bias_k", [P, H], F32)
    inp("biasv", [P, D], BF16)
    inp("gkv", [P, D], BF16)
    inp("bkv", [P, D], BF16)
    inp("cs", [P, 4, RPC], BF16)
    inp("masks", [P, 16, P], BF16)
    t_out = {
        "out": nc.dram_tensor("out", [RPC, D], F32, kind="ExternalOutput"),
        "ckv": nc.dram_tensor("ckv", [RPC, D], BF16, kind="ExternalOutput"),
    }
    with tile.TileContext(nc) as tc:
        _emit(nc, tc, t_in, t_out)
    nc.finalize()
    _CACHE[key] = nc
    return nc


# ---------------------------------------------------------------- host


def _pack_w(w):
    """[D, n] -> [128, 16, n]: tile the contraction dim over (kt, p)."""
    n = w.shape[1]
    return np.ascontiguousarray(
        w.reshape(16, P, n).transpose(1, 0, 2)
    ).astype(NP_BF16)


def host_prep(inputs):
    """Build the 8 per-core input maps (numpy) from full inputs."""
    x = np.asarray(inputs["x"], np.float32).reshape(B * S, D)
    wdq = np.asarray(inputs["W_dq"], np.float32)
    wuq = np.asarray(inputs["W_uq"], np.float32)
    wdkv = np.asarray(inputs["W_dkv"], np.float32)
    wukv = np.asarray(inputs["W_ukv"], np.float32)
    wot = np.asarray(inputs["W_o"], np.float32).T
    gq = np.asarray(inputs["q_gamma"], np.float32)
    bq = np.asarray(inputs["q_beta"], np.float32)
    gkv = np.asarray(inputs["kv_gamma"], np.float32)
    bkv = np.asarray(inputs["kv_beta"], np.float32)

    # fold LN gamma into up-proj weights; beta becomes an output-channel bias
    wuq_eff = gq[:, None] * wuq
    wukv_eff = gkv[:, None] * wukv
    bias_q = (bq @ wuq).reshape(16, P).T.astype(np.float32)        # [dh, h]
    bias_kv = bkv @ wukv
    bias_k = bias_kv[:D].reshape(16, P).T.astype(np.float32)       # [dh, h]
    biasv = np.ascontiguousarray(
        np.broadcast_to(bias_kv[D:], (P, D))
    ).astype(NP_BF16)

    wdq_pk = _pack_w(wdq)
    wdkv_pk = _pack_w(wdkv)
    wuq_pk = _pack_w(wuq_eff)
    wk_pk = _pack_w(wukv_eff[:, :D])
    wv_pk = _pack_w(wukv_eff[:, D:])
    wot_pk = _pack_w(wot)

    gkv_bc = np.ascontiguousarray(np.broadcast_to(gkv, (P, D))).astype(NP_BF16)
    bkv_bc = np.ascontiguousarray(np.broadcast_to(bkv, (P, D))).astype(NP_BF16)

    freqs = 1.0 / (ROPE_THETA ** (np.arange(0, DH, 2, dtype=np.float32) / DH))
    t = np.arange(S, dtype=np.float32)
    emb = np.outer(t, freqs)                      # [S, 64]
    cos = np.concatenate([np.cos(emb), np.cos(emb)], -1).T  # [128, S]
    sin = np.concatenate([np.sin(emb), np.sin(emb)], -1).T
    sin_signed = sin.copy()
    sin_signed[:64] *= -1.0
    scale = 1.0 / math.sqrt(DH)

    in_maps = []
    for c in range(NCORES):
        b = c // 4
        blks = _blocks(c)
        rows = np.concatenate(
            [np.arange(blk * P, (blk + 1) * P) for blk in blks]
        )
        pos = rows  # positions within the batch
        x_c = np.ascontiguousarray(x[b * S + rows]).astype(NP_BF16)
        cs_pack = np.stack(
            [cos[:, pos], sin_signed[:, pos], cos[:, pos] * scale,
             sin_signed[:, pos] * scale], axis=1
        ).astype(NP_BF16)                                   # [128, 4, 512]
        # {0,1} masks for the 16 (rank, slot) same-slot tiles
        mask = np.zeros((P, 16, P), np.float32)
        for ro in range(4):
            oblk = _blocks(4 * b + ro)   # blocks of rank ro in my group
            for sl in range(4):
                bk = oblk[sl]
                bqk = blks[sl]
                kpos = bk * P + np.arange(P)[:, None]
                qpos = bqk * P + np.arange(P)[None, :]
                mask[:, ro * 4 + sl, :] = (kpos <= qpos)
        in_maps.append(
            {
                "x": x_c,
                "wdq": wdq_pk, "wdkv": wdkv_pk, "wuq": wuq_pk,
                "wk": wk_pk, "wv": wv_pk, "wot": wot_pk,
                "bias_q": bias_q, "bias_k": bias_k, "biasv": biasv,
                "gkv": gkv_bc, "bkv": bkv_bc,
                "cs": cs_pack,
                "masks": mask.astype(NP_BF16),
            }
        )
    return in_maps


def host_unshard(results):
    out = np.zeros((B * S, D), np.float32)
    ckv = np.zeros((B * S, D), np.float32)
    for c in range(NCORES):
        b = c // 4
        for qs, blk in enumerate(_blocks(c)):
            g = b * S + blk * P
            out[g : g + P] = results[c]["out"][qs * P : (qs + 1) * P]
            ckv[g : g + P] = results[c]["ckv"][qs * P : (qs + 1) * P].astype(
                np.float32
            )
    return out.reshape(B, S, D), ckv.reshape(B, S, D)


def kernel(**inputs):
    nc = _build()
    in_maps = host_prep(inputs)
    res = run_bass_kernel_spmd(nc, in_maps, core_ids=list(range(NCORES)))
    return host_unshard(res.results)


if __name__ == "__main__":
    rng = np.random.default_rng(0)
    ins = {
        "x": rng.standard_normal((B, S, D), np.float32),
        "W_dq": 0.02 * rng.standard_normal((D, D), np.float32),
        "W_uq": 0.02 * rng.standard_normal((D, D), np.float32),
        "q_gamma": np.ones(D, np.float32),
        "q_beta": np.zeros(D, np.float32),
        "W_dkv": 0.02 * rng.standard_normal((D, D), np.float32),
        "W_ukv": 0.02 * rng.standard_normal((D, 2 * D), np.float32),
        "kv_gamma": np.ones(D, np.float32),
        "kv_beta": np.zeros(D, np.float32),
        "W_o": 0.02 * rng.standard_normal((D, D), np.float32),
    }
    o, ck = kernel(**ins)
    print(o.shape, ck.shape, float(np.abs(o).mean()), float(np.abs(ck).mean()))
